# revision 23
# baseline (speedup 1.0000x reference)
"""Trainium2 Bass kernel for nn_CrossAttentionFusion.

Sharding: 8 cores = (batch b in 0..4) x (image-row half in 0..2).
Each core computes cross-attention for its 2048 query pixels (K/V over the
full 4096 pixels of its batch element, K/V compute replicated within the
pair), then the conv+BN+relu stack for its half of the image.  BatchNorm
batch statistics and conv halo rows are exchanged with AllGather
collectives (packed into single contiguous DMAs via SBUF staging).

Layouts (per core):
  feature maps     F   [p=128, kt=2, n=4096]  bf16 (host pre-shuffled +
                   pre-rolled so this core's query rows are cols 0:2048)
  projections      qT  [c, n=2048], kT [c, m=4096]   (channel-major)
                   V   [m=4096 (32 tiles), c=256]    (pixel-major)
  scores (ST form) ST[m-tile, n-chunk] = sum_c kT[c,m] qT[c,n] -> exp ->
                   Z[n] += ones.T @ E,  fused[c,n] += V.T @ E   (all on PE)
                   1/Z broadcast to 128 partitions via K=1 ones matmul
  conv input       X  [ci, 34, 66] zero-padded spatially, bf16
"""

import os
import numpy as np
import ml_dtypes

N_CORES = 8
B, C, H, W = 4, 256, 64, 64
HW = H * W              # 4096
NQ = 2048               # queries per core
CH = 512                # free-dim chunk (1 PSUM bank of f32)
NCH = NQ // CH          # 4
MT = HW // 128          # 32 m-tiles
EPS = 1e-5
PADW = 66
PADR = 34
NPAD = PADR * PADW      # 2244
ROWSZ = 2 * 64 + 2      # per-channel payload in conv stat/halo exchange

# conv layer specs: (Cin, Cout, cin_tiles, cout_tiles)
CONVS = [(512, 256, 4, 2), (256, 128, 2, 1), (128, 64, 1, 1)]

_CACHE = {}


# --------------------------------------------------------------------------
# walrus in this container rejects >1 sync wait per instruction; split extras
# onto same-engine NOPs placed immediately before the offending instruction.
def _split_excess_waits(nc, max_waits=1):
    import bass_rust
    eng_map = dict(nc.engines)
    n_split = 0
    for fn in nc.m.functions:
        for bb in fn.blocks:
            lst = bb.instructions
            i = 0
            while i < len(lst):
                inst = lst[i]
                si = inst.sync_info
                if si is not None and len(si.on_wait) > max_waits:
                    waits = list(si.on_wait)
                    keep = waits[-max_waits:]
                    extra = waits[:-max_waits]
                    inst.sync_info = bass_rust.SyncInfo(
                        on_wait=keep, on_update=list(si.on_update))
                    eng = eng_map[inst.engine]
                    pos = i
                    for j in range(0, len(extra), max_waits):
                        chunk = extra[j:j + max_waits]
                        bi = eng.nop()
                        nop_inst = bi.ins
                        cur = nc.cur_bb.bb.instructions
                        assert cur[-1] is nop_inst
                        cur.pop()
                        nop_inst.sync_info = bass_rust.SyncInfo(
                            on_wait=chunk, on_update=[])
                        lst.insert(pos, nop_inst)
                        pos += 1
                        i += 1
                    n_split += 1
                i += 1
    return n_split


def _build_program(debug=False):
    import concourse.bass as bass
    import concourse.mybir as mybir
    import concourse.tile as tile

    f32 = mybir.dt.float32
    bf16 = mybir.dt.bfloat16
    FT = mybir.ActivationFunctionType
    AX = mybir.AxisListType

    nc = bass.Bass()

    # ---------------- DRAM I/O (all host pre-shuffled, contiguous) --------
    d_f1 = nc.dram_tensor("f1", [128, 2 * HW], bf16, kind="ExternalInput")
    d_f2 = nc.dram_tensor("f2", [128, 2 * HW], bf16, kind="ExternalInput")
    d_w = {}
    d_b = {}
    for nm in ("q1", "k2", "v2", "q2", "k1", "v1"):
        d_w[nm] = nc.dram_tensor(f"w_{nm}", [128, 2 * C], bf16,
                                 kind="ExternalInput")
    for nm in ("q1", "k2", "q2", "k1"):
        d_b[nm] = nc.dram_tensor(f"b_{nm}", [128, 2], f32,
                                 kind="ExternalInput")
    d_bv1r = nc.dram_tensor("bv1r", [128, C], f32, kind="ExternalInput")
    d_bv2r = nc.dram_tensor("bv2r", [128, C], f32, kind="ExternalInput")

    d_wc, d_bn = [], []
    for li, (cin, cout, cit_n, cot_n) in enumerate(CONVS):
        d_wc.append(nc.dram_tensor(f"wc{li}", [128, cit_n * 9 * cout], bf16,
                                   kind="ExternalInput"))
        parts = min(cout, 128)
        # bc, gamma, beta stacked: [parts, 3*cot_n]
        d_bn.append(nc.dram_tensor(f"bn{li}", [parts, 3 * cot_n], f32,
                                   kind="ExternalInput"))

    d_sel0 = nc.dram_tensor("sel0", [128, 2, 64, 8], bf16, kind="ExternalInput")
    d_self = nc.dram_tensor("selF", [128, 2, 64, 8], f32, kind="ExternalInput")
    d_rowm = nc.dram_tensor("rowm", [128, 2, 64], bf16, kind="ExternalInput")

    d_out = nc.dram_tensor("yout", [64, 32 * 64], f32, kind="ExternalOutput")

    # collective buffers
    SZ0 = 512                       # bf16 elems per partition: [t,cit,w]
    cc0i = nc.dram_tensor("cc0i", [128, SZ0], bf16)
    cc0o = nc.dram_tensor("cc0o", [8, 128, SZ0], bf16, addr_space="Shared")
    ccLi, ccLo = [], []
    for li, (cin, cout, cit_n, cot_n) in enumerate(CONVS):
        parts = min(cout, 128)
        ccLi.append([nc.dram_tensor(f"cc{li+1}i{c_}", [parts, ROWSZ], f32)
                     for c_ in range(cot_n)])
        ccLo.append([nc.dram_tensor(f"cc{li+1}o{c_}", [8, parts, ROWSZ], f32,
                                    addr_space="Shared")
                     for c_ in range(cot_n)])

    dbg = {}
    if debug:
        dbg["qT1"] = nc.dram_tensor("d_qT1", [128, 2, NQ], bf16, kind="ExternalOutput")
        dbg["kT2"] = nc.dram_tensor("d_kT2", [128, 2, HW], bf16, kind="ExternalOutput")
        dbg["V2"] = nc.dram_tensor("d_V2", [128, MT, C], bf16, kind="ExternalOutput")
        dbg["Z"] = nc.dram_tensor("d_Z", [2, NCH, CH], f32, kind="ExternalOutput")
        dbg["X1"] = nc.dram_tensor("d_X1", [128, 4, NPAD], bf16, kind="ExternalOutput")
        dbg["Y1"] = nc.dram_tensor("d_Y1", [2, 128, NQ], f32, kind="ExternalOutput")
        dbg["SC1"] = nc.dram_tensor("d_SC1", [2, 128, 2], f32, kind="ExternalOutput")
        dbg["X2"] = nc.dram_tensor("d_X2", [128, 2, NPAD], bf16, kind="ExternalOutput")

    replica = [list(range(N_CORES))]

    with tile.TileContext(nc) as tc:
        with (
            tc.tile_pool(name="consts", bufs=1) as consts,
            tc.tile_pool(name="xpads", bufs=1) as xpads,
            tc.tile_pool(name="stage", bufs=2) as stage,
        ):
            # ---- constants (contiguous loads, spread across queues) ----
            ones = consts.tile([128, 1], bf16, name="ones")
            nc.vector.memset(ones, 1.0)
            ones1 = consts.tile([1, 128], f32, name="ones1")
            nc.vector.memset(ones1, 1.0)

            w_sb = {}
            for nm in ("q1", "k2", "v2", "q2", "k1", "v1"):
                t = consts.tile([128, 2, C], bf16, name=f"w_{nm}", tag=f"w_{nm}")
                nc.sync.dma_start(out=t[:], in_=d_w[nm][:])
                w_sb[nm] = t
            b_sb = {}
            for nm in ("q1", "k2", "q2", "k1"):
                t = consts.tile([128, 2], f32, name=f"b_{nm}", tag=f"b_{nm}")
                nc.sync.dma_start(out=t[:], in_=d_b[nm][:])
                b_sb[nm] = t
            bv1r = consts.tile([128, C], f32, name="bv1r")
            nc.sync.dma_start(out=bv1r[:], in_=d_bv1r[:])
            bv2r = consts.tile([128, C], f32, name="bv2r")
            nc.sync.dma_start(out=bv2r[:], in_=d_bv2r[:])

            wc_sb = []
            bn_sb = []
            for li, (cin, cout, cit_n, cot_n) in enumerate(CONVS):
                parts = min(cout, 128)
                t = consts.tile([128, cit_n, 3, 3, cout], bf16,
                                name=f"wc{li}", tag=f"wc{li}")
                nc.gpsimd.dma_start(out=t[:], in_=d_wc[li][:])
                wc_sb.append(t)
                t2 = consts.tile([parts, 3, cot_n], f32, name=f"bn{li}",
                                 tag=f"bn{li}")
                nc.gpsimd.dma_start(out=t2[:], in_=d_bn[li][:])
                bn_sb.append(t2)

            sel0 = consts.tile([128, 2, 64, 8], bf16, name="sel0")
            nc.gpsimd.dma_start(out=sel0[:], in_=d_sel0[:])
            selF = consts.tile([128, 2, 64, 8], f32, name="selF")
            nc.gpsimd.dma_start(out=selF[:], in_=d_self[:])
            rowm = consts.tile([128, 2, 64], bf16, name="rowm")
            nc.gpsimd.dma_start(out=rowm[:], in_=d_rowm[:])

            # ---- persistent activations ----
            X1 = xpads.tile([128, 4, NPAD], bf16, name="X1")
            nc.vector.memset(X1, 0.0)

            qkv_ctx = tc.tile_pool(name="qkv", bufs=1)
            qkv = qkv_ctx.__enter__()
            qT = [qkv.tile([128, 2, NQ], bf16, name="qT1", tag="qT1"),
                  qkv.tile([128, 2, NQ], bf16, name="qT2", tag="qT2")]
            kT = [qkv.tile([128, 2, HW], bf16, name="kT2", tag="kT2"),
                  qkv.tile([128, 2, HW], bf16, name="kT1", tag="kT1")]
            Vv = [qkv.tile([128, MT, C], bf16, name="V2", tag="V2"),
                  qkv.tile([128, MT, C], bf16, name="V1", tag="V1")]

            # =========== PHASE 1: projections ===========
            with (
                tc.tile_pool(name="fmaps", bufs=1) as fmaps,
                tc.tile_pool(name="ppsum", bufs=4, space="PSUM") as ppsum,
            ):
                F1 = fmaps.tile([128, 2, HW], bf16, name="F1")
                F2 = fmaps.tile([128, 2, HW], bf16, name="F2")
                for kt_ in range(2):
                    for lo, hi in ((0, 1024), (1024, HW)):
                        nc.scalar.dma_start(
                            out=F1[:, kt_, lo:hi],
                            in_=d_f1[:, kt_ * HW + lo:kt_ * HW + hi])
                        nc.gpsimd.dma_start(
                            out=F2[:, kt_, lo:hi],
                            in_=d_f2[:, kt_ * HW + lo:kt_ * HW + hi])

                # qT / kT projections: out[c,n] = sum_i wT[i,c] F[i,n] + b[c]
                for dst, wname, Fsrc, ncols in (
                    (qT[0], "q1", F1, NQ), (qT[1], "q2", F2, NQ),
                    (kT[0], "k2", F2, HW), (kT[1], "k1", F1, HW),
                ):
                    wt = w_sb[wname]
                    for ct in range(2):
                        for ch in range(ncols // CH):
                            ps = ppsum.tile([128, CH], f32, name="projps",
                                            tag="projps")
                            for kt in range(2):
                                nc.tensor.matmul(
                                    ps[:], wt[:, kt, ct * 128:(ct + 1) * 128],
                                    Fsrc[:, kt, ch * CH:(ch + 1) * CH],
                                    start=(kt == 0), stop=(kt == 1))
                            bias_ap = b_sb[wname][:, ct:ct + 1]
                            nc.scalar.activation(
                                out=dst[:, ct, ch * CH:(ch + 1) * CH],
                                in_=ps[:], func=FT.Identity, bias=bias_ap)
                # V projections: out[m,c] = sum_i F[i,m] wT[i,c] + bv[c]
                for dst, wname, Fsrc, bvr in (
                    (Vv[0], "v2", F2, bv2r), (Vv[1], "v1", F1, bv1r),
                ):
                    wt = w_sb[wname]
                    for mt in range(MT):
                        ps = ppsum.tile([128, C], f32, name="vps", tag="vps")
                        for kt in range(2):
                            nc.tensor.matmul(
                                ps[:], Fsrc[:, kt, mt * 128:(mt + 1) * 128],
                                wt[:, kt, :], start=(kt == 0), stop=(kt == 1))
                        nc.vector.tensor_add(dst[:, mt, :], ps[:], bvr[:])

            if debug:
                nc.sync.dma_start(out=dbg["qT1"][:], in_=qT[0][:])
                nc.sync.dma_start(out=dbg["kT2"][:], in_=kT[0][:])
                nc.sync.dma_start(out=dbg["V2"][:], in_=Vv[0][:])

            # =========== PHASE 2: attention ===========
            # chunk order [0,3,1,2]: boundary rows (chunks 0,3) finish first
            # so the fused-halo collective overlaps the remaining chunks.
            with (
                tc.tile_pool(name="pvps", bufs=4, space="PSUM") as pvps,
                tc.tile_pool(name="zps", bufs=2, space="PSUM") as zps,
                tc.tile_pool(name="stps", bufs=2, space="PSUM") as stps,
                tc.tile_pool(name="attnw", bufs=2) as attnw,
                tc.tile_pool(name="epool", bufs=4) as epool,
                tc.tile_pool(name="ghw", bufs=2) as ghw,
            ):
                def attn_chunk(a, ch):
                    Q, K, V = qT[a], kT[a], Vv[a]
                    pv = [pvps.tile([128, CH], f32, name=f"pv{c_}",
                                    tag="pv") for c_ in range(2)]
                    z = zps.tile([1, CH], f32, name="z", tag="z")

                    def zpv(pmt, e):
                        nc.tensor.matmul(z[:], ones[:], e[:],
                                         start=(pmt == 0),
                                         stop=(pmt == MT - 1))
                        for ct in range(2):
                            nc.tensor.matmul(
                                pv[ct][:],
                                V[:, pmt, ct * 128:(ct + 1) * 128], e[:],
                                start=(pmt == 0), stop=(pmt == MT - 1))

                    pend = []
                    for mt in range(MT):
                        st = stps.tile([128, CH], f32, name="st", tag="st")
                        for kt in range(2):
                            nc.tensor.matmul(
                                st[:], K[:, kt, mt * 128:(mt + 1) * 128],
                                Q[:, kt, ch * CH:(ch + 1) * CH],
                                start=(kt == 0), stop=(kt == 1))
                        e = epool.tile([128, CH], bf16, name="e", tag="e")
                        nc.scalar.activation(out=e[:], in_=st[:],
                                             func=FT.Exp, scale=1.0 / 16.0)
                        pend.append((mt, e))
                        if len(pend) > 2:
                            pmt, pe_ = pend.pop(0)
                            zpv(pmt, pe_)
                    for pmt, pe_ in pend:
                        zpv(pmt, pe_)
                    # epilogue: broadcast Z to 128 partitions on PE, then
                    # reciprocal + scale on DVE.  No DMA involved.
                    zsb = attnw.tile([1, CH], f32, name="zsb", tag="zsb")
                    nc.vector.tensor_copy(zsb[:], z[:])
                    bz = zps.tile([128, CH], f32, name="bzp", tag="z")
                    nc.tensor.matmul(bz[:], ones1[:], zsb[:],
                                     start=True, stop=True)
                    rbz = attnw.tile([128, CH], f32, name="rbz", tag="rbz")
                    nc.vector.reciprocal(rbz[:], bz[:])
                    if debug:
                        nc.sync.dma_start(out=dbg["Z"][a, ch],
                                          in_=rbz[0:1, :])
                    for ct in range(2):
                        cit = 2 * a + ct
                        xv = X1[:, cit].rearrange("p (r c) -> p r c", c=PADW)
                        nc.vector.tensor_mul(
                            xv[:, 1 + ch * 8:1 + ch * 8 + 8, 1:65],
                            pv[ct][:].rearrange("p (r w) -> p r w", w=64),
                            rbz[:].rearrange("p (r w) -> p r w", w=64))

                def pack_cc0():
                    st0 = stage.tile([128, 2, 4, 64], bf16, name="st0",
                                     tag="st0")
                    for t_, pr in ((0, 1), (1, 32)):
                        for cit in range(4):
                            xv = X1[:, cit].rearrange("p (r c) -> p r c",
                                                      c=PADW)
                            nc.scalar.copy(st0[:, t_, cit],
                                           xv[:, pr, 1:65])
                    nc.sync.dma_start(out=cc0i[:], in_=st0[:])
                    nc.gpsimd.collective_compute(
                        "AllGather", mybir.AluOpType.bypass,
                        ins=[cc0i[:]], outs=[cc0o[:]],
                        replica_groups=replica)

                for ch in (0, 3, 1, 2):
                    for a in range(2):
                        attn_chunk(a, ch)
                    if ch == 1:
                        pack_cc0()

                # unpack ghosts: one contiguous gather DMA, DVE select
                G0 = ghw.tile([128, 8, SZ0], bf16, name="G0")
                ap = cc0o[:]
                nc.sync.dma_start(out=G0[:], in_=bass.AP(
                    tensor=ap.tensor, offset=0,
                    ap=[[SZ0, 128], [128 * SZ0, 8], [1, SZ0]]))
                for cit in range(4):
                    for td, ts_ in ((0, 1), (1, 0)):
                        gap = G0[:]
                        gsl = bass.AP(tensor=gap.tensor,
                                      offset=gap.offset + ts_ * 256 + cit * 64,
                                      ap=[list(gap.ap[0]), [1, 64], [SZ0, 8]])
                        prod = ghw.tile([128, 64, 8], bf16, name="prod0",
                                        tag="prod0")
                        nc.vector.tensor_mul(prod[:], gsl, sel0[:, td])
                        g = ghw.tile([128, 64], f32, name="g0", tag="g0")
                        nc.vector.reduce_sum(g[:], prod[:], axis=AX.X)
                        xv = X1[:, cit].rearrange("p (r c) -> p r c", c=PADW)
                        pr = 0 if td == 0 else 33
                        nc.vector.tensor_copy(xv[:, pr, 1:65], g[:])

            qkv_ctx.__exit__(None, None, None)

            if debug:
                nc.sync.dma_start(out=dbg["X1"][:], in_=X1[:])

            # =========== PHASE 3: conv stack ===========
            Xcur = X1
            with (
                tc.tile_pool(name="cpsum", bufs=4, space="PSUM") as cpsum,
                tc.tile_pool(name="convw", bufs=2) as convw,
            ):
                for li, (cin, cout, cit_n, cot_n) in enumerate(CONVS):
                    parts = min(cout, 128)
                    wct = wc_sb[li]
                    bnt = bn_sb[li]  # [parts, 3(bc,g,bb), cot_n]
                    last = li == len(CONVS) - 1
                    if not last:
                        Xnext = xpads.tile([128, cot_n, NPAD], bf16,
                                           name=f"X{li+2}", tag=f"X{li+2}")
                        nc.vector.memset(Xnext, 0.0)
                    yf = [convw.tile([parts, NQ], f32, name=f"y{li}_{cot}",
                                     tag=f"y{li}_{cot}", bufs=1)
                          for cot in range(cot_n)]
                    for cot in range(cot_n):
                        stg = stage.tile([parts, ROWSZ], f32,
                                         name=f"stgL{li}_{cot}", tag="stgL")
                        pss = {}
                        for ch in (1, 2, 0, 3):
                            pss[ch] = cpsum.tile([parts, CH], f32,
                                                 name=f"cps{ch}", tag="cps")
                        for cit in range(cit_n):
                            xv = Xcur[:, cit].rearrange(
                                "p (r c) -> p r c", c=PADW)
                            for ch in (1, 2, 0, 3):
                                for dy in range(3):
                                    for dx in range(3):
                                        nc.tensor.matmul(
                                            pss[ch][:],
                                            wct[:, cit, dy, dx,
                                                cot * 128:cot * 128 + parts],
                                            xv[:, ch * 8 + dy:ch * 8 + dy + 8,
                                               dx:dx + 64],
                                            start=(cit == 0 and dy == 0
                                                   and dx == 0),
                                            stop=(cit == cit_n - 1 and dy == 2
                                                  and dx == 2))
                        for ch in (1, 2, 0, 3):
                            nc.scalar.activation(
                                out=yf[cot][:, ch * CH:(ch + 1) * CH],
                                in_=pss[ch][:], func=FT.Identity,
                                bias=bnt[:, 0, cot:cot + 1])
                        # local stats for this cot
                        bns = convw.tile([parts, 4, 6], f32, name="bns",
                                         tag="bns")
                        for sg in range(4):
                            nc.vector.bn_stats(
                                out=bns[:, sg],
                                in_=yf[cot][:, sg * CH:(sg + 1) * CH])
                        mv = convw.tile([parts, 2], f32, name="mv", tag="mv")
                        nc.vector.bn_aggr(out=mv[:], in_=bns[:])
                        # sum = mean*2048 ; sumsq = (var + mean^2)*2048
                        nc.vector.tensor_scalar_mul(
                            stg[:, 128:129], mv[:, 0:1], float(NQ))
                        m2 = convw.tile([parts, 1], f32, name="m2", tag="m2")
                        nc.vector.tensor_mul(m2[:], mv[:, 0:1], mv[:, 0:1])
                        nc.vector.tensor_add(stg[:, 129:130],
                                             mv[:, 1:2], m2[:])
                        nc.vector.tensor_scalar_mul(
                            stg[:, 129:130], stg[:, 129:130],
                            float(NQ))
                        # boundary rows into staging
                        nc.vector.tensor_copy(stg[:, 0:64],
                                              yf[cot][:, 0:64])
                        nc.vector.tensor_copy(stg[:, 64:128],
                                              yf[cot][:, NQ - 64:NQ])
                        nc.sync.dma_start(out=ccLi[li][cot][:], in_=stg[:])
                        nc.gpsimd.collective_compute(
                            "AllGather", mybir.AluOpType.bypass,
                            ins=[ccLi[li][cot][:]], outs=[ccLo[li][cot][:]],
                            replica_groups=replica)

                    for cot in range(cot_n):
                        GL = convw.tile([parts, 8, ROWSZ], f32,
                                        name=f"GL{li}_{cot}", tag="GL")
                        ap = ccLo[li][cot][:]
                        nc.sync.dma_start(out=GL[:], in_=bass.AP(
                            tensor=ap.tensor, offset=0,
                            ap=[[ROWSZ, parts], [parts * ROWSZ, 8],
                                [1, ROWSZ]]))
                        gap = GL[:]
                        # stats: [parts, (t:2), (j:8)] -> reduce over j
                        ssl = bass.AP(tensor=gap.tensor,
                                      offset=gap.offset + 128,
                                      ap=[list(gap.ap[0]), [1, 2],
                                          [ROWSZ, 8]])
                        tot = convw.tile([parts, 2], f32, name="tot",
                                         tag="tot")
                        nc.vector.reduce_sum(tot[:], ssl, axis=AX.X)
                        mn = convw.tile([parts, 1], f32, name="mn", tag="mn")
                        nc.vector.tensor_scalar_mul(mn[:], tot[:, 0:1],
                                                    1.0 / 16384.0)
                        ex2 = convw.tile([parts, 1], f32, name="ex2",
                                         tag="ex2")
                        nc.vector.tensor_scalar_mul(ex2[:], tot[:, 1:2],
                                                    1.0 / 16384.0)
                        m2b = convw.tile([parts, 1], f32, name="m2b",
                                         tag="m2b")
                        nc.vector.tensor_mul(m2b[:], mn[:], mn[:])
                        var = convw.tile([parts, 1], f32, name="var",
                                         tag="var")
                        nc.vector.tensor_sub(var[:], ex2[:], m2b[:])
                        nc.vector.tensor_scalar_add(var[:], var[:], EPS)
                        lnv = convw.tile([parts, 1], f32, name="lnv",
                                         tag="lnv")
                        nc.scalar.activation(out=lnv[:], in_=var[:],
                                             func=FT.Ln)
                        rstd = convw.tile([parts, 1], f32, name="rstd",
                                          tag="rstd")
                        nc.scalar.activation(out=rstd[:], in_=lnv[:],
                                             func=FT.Exp, scale=-0.5)
                        scl = convw.tile([parts, 1], f32, name="scl",
                                         tag="scl")
                        nc.vector.tensor_mul(scl[:], bnt[:, 1, cot:cot + 1],
                                             rstd[:])
                        bia = convw.tile([parts, 1], f32, name="bia",
                                         tag="bia")
                        nc.vector.tensor_mul(bia[:], mn[:], scl[:])
                        nc.vector.tensor_sub(bia[:], bnt[:, 2, cot:cot + 1],
                                             bia[:])
                        if debug and li == 0:
                            nc.sync.dma_start(out=dbg["SC1"][cot, :, 0:1],
                                              in_=scl[:])
                            nc.sync.dma_start(out=dbg["SC1"][cot, :, 1:2],
                                              in_=bia[:])

                        if last:
                            outf = convw.tile([parts, NQ], f32, name="outf",
                                              tag="outf")
                            nc.scalar.activation(
                                out=outf[:], in_=yf[cot][:], func=FT.Relu,
                                scale=scl[:], bias=bia[:])
                            nc.sync.dma_start(out=d_out[:], in_=outf[:])
                        else:
                            xv = Xnext[:, cot].rearrange("p (r c) -> p r c",
                                                         c=PADW)
                            nc.scalar.activation(
                                out=xv[:parts, 1:33, 1:65],
                                in_=yf[cot][:].rearrange("p (r w) -> p r w",
                                                         w=64),
                                func=FT.Relu, scale=scl[:], bias=bia[:])
                            # ghost rows from gathered buffer
                            for td, ts_ in ((0, 1), (1, 0)):
                                gsl = bass.AP(
                                    tensor=gap.tensor,
                                    offset=gap.offset + ts_ * 64,
                                    ap=[list(gap.ap[0]), [1, 64],
                                        [ROWSZ, 8]])
                                prod = convw.tile([parts, 64, 8], f32,
                                                  name="prodL", tag="prodL")
                                nc.vector.tensor_mul(prod[:], gsl,
                                                     selF[:parts, td])
                                graw = convw.tile([parts, 64], f32,
                                                  name="grawL", tag="grawL")
                                nc.vector.reduce_sum(graw[:], prod[:],
                                                     axis=AX.X)
                                gb = convw.tile([parts, 64], bf16,
                                                name="gbL", tag="gbL")
                                nc.scalar.activation(out=gb[:], in_=graw[:],
                                                     func=FT.Relu,
                                                     scale=scl[:], bias=bia[:])
                                pr = 0 if td == 0 else 33
                                nc.vector.tensor_mul(xv[:parts, pr, 1:65],
                                                     gb[:],
                                                     rowm[:parts, td])
                    if debug and li == 0:
                        for cot in range(cot_n):
                            nc.sync.dma_start(out=dbg["Y1"][cot],
                                              in_=yf[cot][:])
                        if not last:
                            nc.sync.dma_start(out=dbg["X2"][:], in_=Xnext[:])
                    if not last:
                        Xcur = Xnext

    n = _split_excess_waits(nc, 1)
    return nc, n


def _shard_inputs(inputs):
    """Build the 8 per-core input maps from the full problem inputs."""
    bf = ml_dtypes.bfloat16
    fm1 = np.asarray(inputs["feature_map1"], np.float32)
    fm2 = np.asarray(inputs["feature_map2"], np.float32)

    def pshuf(a2d):  # [2*128, X] -> [128, 2*X] partition-major
        n2, x = a2d.shape
        kt = n2 // 128
        return np.ascontiguousarray(
            a2d.reshape(kt, 128, x).transpose(1, 0, 2).reshape(128, kt * x))

    shared = {}
    for nm in ("q1", "k2", "v2", "q2", "k1", "v1"):
        wT = np.asarray(inputs[f"{nm}_w"], np.float32).T  # [in, out]
        shared[f"w_{nm}"] = pshuf(wT).astype(bf)
    for nm in ("q1", "k2", "q2", "k1"):
        b = np.asarray(inputs[f"{nm}_b"], np.float32)
        shared[f"b_{nm}"] = np.ascontiguousarray(b.reshape(2, 128).T)
    shared["bv1r"] = np.tile(np.asarray(inputs["v1_b"], np.float32)[None, :],
                             (128, 1))
    shared["bv2r"] = np.tile(np.asarray(inputs["v2_b"], np.float32)[None, :],
                             (128, 1))
    for li, (cin, cout, cit_n, cot_n) in enumerate(CONVS):
        wc = np.asarray(inputs[f"conv{li+1}_w"], np.float32)  # [co, ci, 3, 3]
        # -> [p, cit, ky, kx, co]
        arr = wc.transpose(1, 2, 3, 0).reshape(cit_n, 128, 3, 3, cout)
        arr = arr.transpose(1, 0, 2, 3, 4).reshape(128, -1)
        shared[f"wc{li}"] = np.ascontiguousarray(arr).astype(bf)
        parts = min(cout, 128)
        cot_nn = cout // parts
        trio = np.stack([
            np.asarray(inputs[f"conv{li+1}_b"], np.float32),
            np.asarray(inputs[f"bn{li+1}_g"], np.float32),
            np.asarray(inputs[f"bn{li+1}_b"], np.float32),
        ])  # [3, cout]
        # -> [parts, 3, cot_n] -> [parts, 3*cot_n]
        arr = trio.reshape(3, cot_nn, parts).transpose(2, 0, 1)
        shared[f"bn{li}"] = np.ascontiguousarray(arr.reshape(parts, -1))

    in_maps = []
    for r in range(N_CORES):
        b, half = divmod(r, 2)
        h0 = 32 * half
        m = dict(shared)
        # roll rows so this core's query rows are columns 0:2048
        m["f1"] = pshuf(np.roll(fm1[b], -h0, axis=1).reshape(C, HW)).astype(bf)
        m["f2"] = pshuf(np.roll(fm2[b], -h0, axis=1).reshape(C, HW)).astype(bf)
        # ghost row selection: dest td=0 (top ghost) / td=1 (bottom ghost)
        sel = np.zeros((2, 8), np.float32)
        partner = r ^ 1
        if half == 0:
            sel[1, partner] = 1.0   # bottom ghost <- partner's top row
        else:
            sel[0, partner] = 1.0   # top ghost <- partner's bottom row
        selfull = np.broadcast_to(sel[None, :, None, :],
                                  (128, 2, 64, 8)).copy()
        m["sel0"] = selfull.astype(bf)
        m["selF"] = selfull.astype(np.float32)
        rowmask = sel.sum(-1)  # [2]
        m["rowm"] = np.broadcast_to(rowmask[None, :, None],
                                    (128, 2, 64)).copy().astype(bf)
        in_maps.append(m)
    return in_maps


def _get_program(debug=False):
    key = ("dbg" if debug else "rel")
    if key not in _CACHE:
        _CACHE[key] = _build_program(debug=debug)
    return _CACHE[key]


def run(inputs, trace=False, debug=False):
    from concourse.bass_utils import run_bass_kernel_spmd
    nc, _ = _get_program(debug=debug)
    in_maps = _shard_inputs(inputs)
    res = run_bass_kernel_spmd(nc, in_maps, list(range(N_CORES)), trace=trace)
    out = np.zeros((B, 64, H, W), np.float32)
    for r in range(N_CORES):
        b, half = divmod(r, 2)
        h0 = 32 * half
        out[b, :, h0:h0 + 32, :] = res.results[r]["yout"].reshape(64, 32, 64)
    return out, res


def kernel(**inputs):
    out, _ = run(inputs, trace=False)
    return out
